# revision 1
# baseline (speedup 1.0000x reference)
"""Trainium2 Bass kernel for nn_Attention_51634096833229.

Conv-projection attention block (CvT-style): depthwise 3x3 conv + BN on the
28x28 token image for each of q/k/v, linear qkv projections, 3-head attention
over 785 tokens (784 image + 1 cls), output projection.

Sharding: data-parallel over batch, B=32 -> 4 samples per core on 8 cores.

Per-core dataflow (per sample):
  x [785,192] --DMA--> SBUF token-major --PE transpose--> xT [192,785]
  xT image part -> zero-padded [c,30,30] buffer (one 3D-AP copy per chunk)
  dw-conv+BN: 9 fused MAC ops per channel-chunk on DVE (BN folded into taps),
    last tap writes bf16 y; cls column copied from xT
  q,k: feature-major matmul (lhsT = w_qkv^T chunks)  -> qT,kT [192,785] bf16
  v:   token-major matmul (lhsT = y chunks)          -> v [t,192] -> per-head
       vaug [t,65] with ones column (row sums for softmax denominator)
  scores^T [t,l] = kT_h^T qT_h on PE; exp on ACT (scale folded, no max
    subtraction -- |scores| < 3); PV: outT_h[d,l] accumulated over t-chunks,
    row 64 = softmax denominators
  normalize with DVE reciprocal + partition-broadcast multiply -> aT [192,785]
  final: token-major matmul (lhsT = aT chunks, rhs = w_proj^T; bias via
    ones-row augmentation) -> out [t,192] --DMA--> DRAM
"""

import sys

sys.path.insert(0, "/opt/trn_rl_repo")

import numpy as np
import ml_dtypes

import concourse.bass as bass
import concourse.mybir as mybir
import concourse.tile as tile
from concourse import bacc
from concourse.masks import make_identity
from concourse.bass_utils import run_bass_kernel_spmd

F32 = mybir.dt.float32
BF16 = mybir.dt.bfloat16
AF = mybir.ActivationFunctionType
OP = mybir.AluOpType

B, T, C, CO, NH, D = 32, 785, 192, 192, 3, 64
HH = WW = 28
NCORES = 8
BPC = B // NCORES  # samples per core
SCALE = float(CO) ** -0.5
BN_EPS = 1e-5

# token blocks of 128 along T
TBLK = [(i * 128, min(128, T - i * 128)) for i in range((T + 127) // 128)]
# channel chunks along C=192
CCH = [(0, 128), (128, 64)]
# N segments within 785 (psum bank = 512 f32)
NSEG = [(0, 512), (512, T - 512)]


def _conv_shift_ap(pad_ap, dy, dx):
    """3D AP view [P, 28, 28] of the padded [P, 30*30] image for tap (dy,dx)."""
    return pad_ap.rearrange("p (y x) -> p y x", y=30, x=30)[
        :, dy:dy + 28, dx:dx + 28]


def _img3(ap):
    """[P, 784] -> [P, 28, 28] view."""
    return ap.rearrange("p (y x) -> p y x", y=28, x=28)


def build_bass():
    nc = bacc.Bacc(None)
    x_d = nc.declare_dram_parameter("x", [BPC, T, C], F32, isOutput=False)
    wqkvT_d = nc.declare_dram_parameter("wqkvT", [3, C, CO], BF16, isOutput=False)
    wconv_d = nc.declare_dram_parameter("wconv", [C, 27], F32, isOutput=False)
    bnt_d = nc.declare_dram_parameter("bnt", [C, 3], F32, isOutput=False)
    wpa_d = nc.declare_dram_parameter("wpa", [C + 1, CO], BF16, isOutput=False)
    out_d = nc.declare_dram_parameter("out", [BPC, T, CO], F32, isOutput=True)

    from contextlib import ExitStack
    with tile.TileContext(nc) as tc, ExitStack() as es:
        consts = es.enter_context(tc.tile_pool(name="consts", bufs=1))
        psA = es.enter_context(tc.tile_pool(name="psA", bufs=3, space="PSUM"))
        psT = es.enter_context(tc.tile_pool(name="psT", bufs=2, space="PSUM"))
        xload = es.enter_context(tc.tile_pool(name="xload", bufs=3))
        xTp = es.enter_context(tc.tile_pool(name="xT", bufs=2))
        padp = es.enter_context(tc.tile_pool(name="pad", bufs=2))
        accp = es.enter_context(tc.tile_pool(name="acc", bufs=2))
        yp = es.enter_context(tc.tile_pool(name="y", bufs=2))
        qkp = es.enter_context(tc.tile_pool(name="qk", bufs=2))
        vap = es.enter_context(tc.tile_pool(name="va", bufs=2))
        ep = es.enter_context(tc.tile_pool(name="E", bufs=3))
        atp = es.enter_context(tc.tile_pool(name="aT", bufs=2))
        op_ = es.enter_context(tc.tile_pool(name="osb", bufs=3))
        smallp = es.enter_context(tc.tile_pool(name="small", bufs=3))
        if True:
            ident = consts.tile([128, 128], F32, tag="ident", name="ident")
            make_identity(nc, ident[:])

            # weights into SBUF, split by channel chunk
            wq_sb = []  # [i][chunk] -> [P, 192]
            for i in range(3):
                row = []
                for ci, (c0, cp) in enumerate(CCH):
                    t = consts.tile([cp, CO], BF16, tag=f"wq{i}{ci}", name=f"wq{i}{ci}")
                    nc.sync.dma_start(t[:], wqkvT_d[i, c0:c0 + cp, :])
                    row.append(t)
                wq_sb.append(row)
            wc_sb, bnt_sb = [], []
            for ci, (c0, cp) in enumerate(CCH):
                t = consts.tile([cp, 27], F32, tag=f"wc{ci}", name=f"wc{ci}")
                nc.sync.dma_start(t[:], wconv_d[c0:c0 + cp, :])
                wc_sb.append(t)
                t2 = consts.tile([cp, 3], F32, tag=f"bnt{ci}", name=f"bnt{ci}")
                nc.sync.dma_start(t2[:], bnt_d[c0:c0 + cp, :])
                bnt_sb.append(t2)
            wpa0 = consts.tile([128, CO], BF16, tag="wpa0", name="wpa0")
            nc.sync.dma_start(wpa0[:], wpa_d[0:128, :])
            wpa1 = consts.tile([65, CO], BF16, tag="wpa1", name="wpa1")
            nc.sync.dma_start(wpa1[:], wpa_d[128:193, :])

            # persistent per-head vaug tiles: ones column written once
            vaug = [[vap.tile([128, 65], BF16, tag=f"va{h}{tb}",
                              name=f"va{h}{tb}")
                     for tb in range(len(TBLK))] for h in range(NH)]
            for h in range(NH):
                for tb, (t0, tn) in enumerate(TBLK):
                    nc.vector.memset(vaug[h][tb][0:tn, 64:65], 1.0)
            aT0 = atp.tile([128, T], BF16, tag="aT0", name="aT0")
            aT1 = atp.tile([65, T], BF16, tag="aT1", name="aT1")
            nc.vector.memset(aT1[64:65, :], 1.0)

            for b in range(BPC):
                # ---- batched load (768 tokens + 17-token tail) ----
                xin = xload.tile([128, 6 * C], F32, tag="xin", name="xin")
                nc.sync.dma_start(
                    xin[:].rearrange("p (n c) -> p n c", n=6, c=C),
                    x_d[b, 0:768, :].rearrange("(n p) c -> p n c", p=128))
                xtl = xload.tile([17, C], F32, tag="xtl", name="xtl")
                nc.sync.dma_start(xtl[:], x_d[b, 768:785, :])
                # ---- PE transpose to xT (2 channel chunks) ----
                xT = [xTp.tile([128, T], F32, tag="xT0", name="xT0"),
                      xTp.tile([64, T], F32, tag="xT1", name="xT1")]
                for tb, (t0, tn) in enumerate(TBLK):
                    xl = (xin[:, tb * C:tb * C + C] if tb < 6 else xtl[:])
                    ps = psT.tile([128, 256], F32, tag="tr", name="tr")
                    nc.tensor.transpose(ps[0:128, 0:tn], xl[0:tn, 0:128],
                                        ident[0:tn, 0:tn])
                    nc.tensor.transpose(ps[0:64, 128:128 + tn],
                                        xl[0:tn, 128:192], ident[0:tn, 0:tn])
                    nc.any.tensor_copy(xT[0][:, t0:t0 + tn], ps[0:128, 0:tn])
                    nc.any.tensor_copy(xT[1][:, t0:t0 + tn],
                                       ps[0:64, 128:128 + tn])

                # ---- padded image (shared by q/k/v convs) ----
                pads = []
                for ci, (c0, cp) in enumerate(CCH):
                    pad = padp.tile([cp, 900], F32, tag=f"pad{ci}", name=f"pad{ci}")
                    nc.vector.memset(pad[:], 0.0)
                    nc.any.tensor_copy(
                        _conv_shift_ap(pad[:], 1, 1),
                        _img3(xT[ci][:, 1:T]))
                    pads.append(pad)

                # ---- depthwise conv + BN -> y (bf16), cls col prepended ----
                ys = []  # [i][chunk]
                for i in range(3):
                    row = []
                    for ci, (c0, cp) in enumerate(CCH):
                        y = yp.tile([cp, T], BF16, tag=f"y{i}{ci}", name=f"y{i}{ci}")
                        acc = accp.tile([cp, 784], F32, tag=f"acc{ci}", name=f"acc{ci}")
                        acc3 = _img3(acc[:])
                        y3 = _img3(y[:, 1:T])
                        for tap in range(9):
                            dy, dx = tap // 3, tap % 3
                            sh = _conv_shift_ap(pads[ci][:], dy, dx)
                            wcol = wc_sb[ci][:, i * 9 + tap:i * 9 + tap + 1]
                            if tap == 0:
                                nc.vector.tensor_scalar(
                                    acc3, sh, wcol, bnt_sb[ci][:, i:i + 1],
                                    OP.mult, OP.add)
                            elif tap < 8:
                                nc.vector.scalar_tensor_tensor(
                                    acc3, sh, wcol, acc3, OP.mult, OP.add)
                            else:
                                nc.vector.scalar_tensor_tensor(
                                    y3, sh, wcol, acc3, OP.mult, OP.add)
                        nc.any.tensor_copy(y[:, 0:1], xT[ci][:, 0:1])
                        row.append(y)
                    ys.append(row)

                # ---- q,k feature-major projections -> qT,kT bf16 ----
                qkT = []  # [i][chunk]
                for i in range(2):
                    row = []
                    for ob, (o0, osz) in enumerate(CCH):
                        ps = psA.tile([128, T], F32, tag="mm", name="mm")
                        for (n0, nn) in NSEG:
                            for ci in range(2):
                                nc.tensor.matmul(
                                    ps[0:osz, n0:n0 + nn],
                                    wq_sb[i][ci][:, o0:o0 + osz],
                                    ys[i][ci][:, n0:n0 + nn],
                                    start=(ci == 0), stop=(ci == 1))
                        dst = qkp.tile([osz, T], BF16, tag=f"qk{i}{ob}", name=f"qk{i}{ob}")
                        nc.any.tensor_copy(dst[:], ps[0:osz, 0:T])
                        row.append(dst)
                    qkT.append(row)

                def head_rows(qk, h):
                    """[64, T] slice of qT/kT chunks for head h."""
                    if h < 2:
                        return qk[0][h * 64:(h + 1) * 64, :]
                    return qk[1][0:64, :]

                # ---- v token-major -> per-head vaug ----
                for tb, (t0, tn) in enumerate(TBLK):
                    ps = psA.tile([128, T], F32, tag="mm", name="mm")
                    for ci in range(2):
                        nc.tensor.matmul(
                            ps[0:tn, 0:CO],
                            ys[2][ci][:, t0:t0 + tn],
                            wq_sb[2][ci][:],
                            start=(ci == 0), stop=(ci == 1))
                    for h in range(NH):
                        nc.any.tensor_copy(vaug[h][tb][0:tn, 0:64],
                                           ps[0:tn, h * 64:(h + 1) * 64])

                # ---- attention per head ----
                for h in range(NH):
                    kh = head_rows(qkT[1], h)
                    qh = head_rows(qkT[0], h)
                    pv = psA.tile([128, T], F32, tag="mm", name="mm")
                    for tb, (t0, tn) in enumerate(TBLK):
                        ss = psA.tile([128, T], F32, tag="mm", name="mm")
                        for (n0, nn) in NSEG:
                            nc.tensor.matmul(
                                ss[0:tn, n0:n0 + nn],
                                kh[:, t0:t0 + tn], qh[:, n0:n0 + nn],
                                start=True, stop=True)
                        e = ep.tile([128, T], BF16, tag="E", name="E")
                        nc.scalar.activation(e[0:tn, 0:T], ss[0:tn, 0:T],
                                             AF.Exp, scale=SCALE)
                        for (n0, nn) in NSEG:
                            nc.tensor.matmul(
                                pv[0:65, n0:n0 + nn],
                                vaug[h][tb][0:tn, 0:65],
                                e[0:tn, n0:n0 + nn],
                                start=(tb == 0), stop=(tb == len(TBLK) - 1))
                    r = smallp.tile([1, T], F32, tag="r", name="r")
                    nc.vector.reciprocal(r[0:1, :], pv[64:65, 0:T])
                    rb = smallp.tile([64, T], F32, tag="rb", name="rb")
                    nc.gpsimd.partition_broadcast(rb[:], r[0:1, :])
                    dst = aT0[h * 64:(h + 1) * 64, :] if h < 2 else aT1[0:64, :]
                    nc.vector.tensor_tensor(
                        dst, pv[0:64, 0:T], rb[:], OP.mult)

                # ---- final projection (bias via ones row) + store ----
                obuf = op_.tile([128, 6 * CO], F32, tag="obuf", name="obuf")
                otl = op_.tile([17, CO], F32, tag="otl", name="otl")
                for tb, (t0, tn) in enumerate(TBLK):
                    fp = psA.tile([128, T], F32, tag="mm", name="mm")
                    nc.tensor.matmul(fp[0:tn, 0:CO], aT0[:, t0:t0 + tn],
                                     wpa0[:], start=True, stop=False)
                    nc.tensor.matmul(fp[0:tn, 0:CO], aT1[:, t0:t0 + tn],
                                     wpa1[:], start=False, stop=True)
                    dst = obuf[:, tb * CO:tb * CO + CO] if tb < 6 else otl[:]
                    nc.any.tensor_copy(dst[0:tn, :], fp[0:tn, 0:CO])
                nc.sync.dma_start(
                    out_d[b, 0:768, :].rearrange("(n p) c -> p n c", p=128),
                    obuf[:].rearrange("p (n c) -> p n c", n=6, c=CO))
                nc.sync.dma_start(out_d[b, 768:785, :], otl[:])
    if not nc.is_finalized():
        nc.finalize()
    return nc


_NC_CACHE = None


def kernel(**inputs):
    global _NC_CACHE
    x = np.asarray(inputs["x"], dtype=np.float32)
    conv_w = np.asarray(inputs["conv_w"], dtype=np.float32)  # [3,C,1,3,3]
    bn_scale = np.asarray(inputs["bn_scale"], dtype=np.float32)
    bn_bias = np.asarray(inputs["bn_bias"], dtype=np.float32)
    bn_mean = np.asarray(inputs["bn_mean"], dtype=np.float32)
    bn_var = np.asarray(inputs["bn_var"], dtype=np.float32)
    w_qkv = np.asarray(inputs["w_qkv"], dtype=np.float32)  # [3,CO,C]
    w_proj = np.asarray(inputs["w_proj"], dtype=np.float32)  # [CO,CO]
    b_proj = np.asarray(inputs["b_proj"], dtype=np.float32)  # [CO]

    # fold BN into conv taps: y = conv(x, w)*s + (b - mu*s)
    s = bn_scale / np.sqrt(bn_var + BN_EPS)  # [3,C]
    wtap = (conv_w[:, :, 0, :, :].reshape(3, C, 9)
            * s[:, :, None]).astype(np.float32)  # [3,C,9]
    # [C, 27] with column i*9+tap
    wconv_h = np.ascontiguousarray(
        wtap.transpose(1, 0, 2).reshape(C, 27))
    bnt_h = np.ascontiguousarray(
        (bn_bias - bn_mean * s).T).astype(np.float32)  # [C,3]
    wqkvT_h = np.ascontiguousarray(
        w_qkv.transpose(0, 2, 1)).astype(ml_dtypes.bfloat16)  # [3,C,CO]
    wpa_h = np.concatenate(
        [w_proj.T, b_proj[None, :]], axis=0).astype(ml_dtypes.bfloat16)

    if _NC_CACHE is None:
        _NC_CACHE = build_bass()
    nc = _NC_CACHE

    xs = x.reshape(NCORES, BPC, T, C)
    in_maps = [
        {"x": np.ascontiguousarray(xs[c]), "wqkvT": wqkvT_h,
         "wconv": wconv_h, "bnt": bnt_h, "wpa": wpa_h}
        for c in range(NCORES)
    ]
    res = run_bass_kernel_spmd(nc, in_maps, list(range(NCORES)), **RUN_KWARGS)
    global LAST_RESULTS
    LAST_RESULTS = res
    out = np.concatenate([np.asarray(r["out"]) for r in res.results], axis=0)
    return out.reshape(B, T, CO).astype(np.float32)


RUN_KWARGS = {}
LAST_RESULTS = None



# revision 4
# speedup vs baseline: 1.0038x; 1.0038x over previous
"""Trainium2 Bass kernel for nn_Attention_51634096833229.

Conv-projection attention block (CvT-style): depthwise 3x3 conv + BN on the
28x28 token image for each of q/k/v, linear qkv projections, 3-head attention
over 785 tokens (784 image + 1 cls), output projection.

Sharding: data-parallel over batch, B=32 -> 4 samples per core on 8 cores.

Per-core dataflow (per sample):
  x bf16 (host-converted, T padded to 800) --DMA-transpose--> xT [192,800]
  bf16 via two XBAR transposes (c 0:128, c 64:192)
  dw-conv+BN in bf16 (DVE 4x mode): zero-padded [c,30,30] canvas, 9 fused
  MACs per channel-chunk, BN folded into taps; cls column from xT
  q,k: feature-major matmul -> qT,kT [192,785] bf16
  v:   token-major matmul -> per-tblk vaug [t,3*65] bf16 with ones cols
  scores^T [t,l] = kT_h^T qT_h; exp on ACT (scale folded, no max sub);
  PV: outT_h [65,l] accumulated over t; row 64 = softmax denominators;
  emission interleaves scores(t+1) ahead of PV(t) to keep PE fed
  normalize: reciprocal_approx_fast + gpsimd partition-broadcast + DVE mult
  final: token-major matmul (lhsT = aT chunks, rhs = w_proj^T with ones-row
  bias) -> out bf16 --DMA--> DRAM, host converts to f32
"""

import sys

sys.path.insert(0, "/opt/trn_rl_repo")

import numpy as np
import ml_dtypes

import concourse.bass as bass
import concourse.mybir as mybir
import concourse.tile as tile
from concourse import bacc
from concourse.bass_utils import run_bass_kernel_spmd

F32 = mybir.dt.float32
BF16 = mybir.dt.bfloat16
AF = mybir.ActivationFunctionType
OP = mybir.AluOpType

B, T, C, CO, NH, D = 32, 785, 192, 192, 3, 64
TP = 800  # host-padded token count (multiple of 16 for DMA transpose)
HH = WW = 28
NCORES = 8
BPC = B // NCORES  # samples per core
SCALE = float(CO) ** -0.5
BN_EPS = 1e-5

# token blocks of 128 along T
TBLK = [(i * 128, min(128, T - i * 128)) for i in range((T + 127) // 128)]
# channel chunks along C=192
CCH = [(0, 128), (128, 64)]
# N segments within 785 (psum bank = 512 f32)
NSEG = [(0, 512), (512, T - 512)]


def _conv_shift_ap(pad_ap, dy, dx):
    """3D AP view [P, 28, 28] of the padded [P, 30*30] image for tap (dy,dx)."""
    return pad_ap.rearrange("p (y x) -> p y x", y=30, x=30)[
        :, dy:dy + 28, dx:dx + 28]


def _img3(ap):
    """[P, 784] -> [P, 28, 28] view."""
    return ap.rearrange("p (y x) -> p y x", y=28, x=28)


def build_bass():
    nc = bacc.Bacc(None)
    x_d = nc.declare_dram_parameter("x", [BPC, TP, C], BF16, isOutput=False)
    wqkvT_d = nc.declare_dram_parameter("wqkvT", [3, C, CO], BF16, isOutput=False)
    wconv_d = nc.declare_dram_parameter("wconv", [C, 27], F32, isOutput=False)
    bnt_d = nc.declare_dram_parameter("bnt", [C, 3], F32, isOutput=False)
    wpa_d = nc.declare_dram_parameter("wpa", [C + 1, CO], BF16, isOutput=False)
    out_d = nc.declare_dram_parameter("out", [BPC, T, CO], BF16, isOutput=True)

    from contextlib import ExitStack
    with tile.TileContext(nc) as tc, ExitStack() as es:
        consts = es.enter_context(tc.tile_pool(name="consts", bufs=1))
        psA = es.enter_context(tc.tile_pool(name="psA", bufs=3, space="PSUM"))
        xTp = es.enter_context(tc.tile_pool(name="xT", bufs=2))
        padp = es.enter_context(tc.tile_pool(name="pad", bufs=2))
        accp = es.enter_context(tc.tile_pool(name="acc", bufs=2))
        yp = es.enter_context(tc.tile_pool(name="y", bufs=2))
        qkp = es.enter_context(tc.tile_pool(name="qk", bufs=2))
        vap = es.enter_context(tc.tile_pool(name="va", bufs=2))
        ep = es.enter_context(tc.tile_pool(name="E", bufs=3))
        atp = es.enter_context(tc.tile_pool(name="aT", bufs=2))
        op_ = es.enter_context(tc.tile_pool(name="osb", bufs=3))
        smallp = es.enter_context(tc.tile_pool(name="small", bufs=3))
        if True:
            # weights into SBUF, split by channel chunk
            wq_sb = []  # [i][chunk] -> [P, 192]
            for i in range(3):
                row = []
                for ci, (c0, cp) in enumerate(CCH):
                    t = consts.tile([cp, CO], BF16, tag=f"wq{i}{ci}", name=f"wq{i}{ci}")
                    nc.sync.dma_start(t[:], wqkvT_d[i, c0:c0 + cp, :])
                    row.append(t)
                wq_sb.append(row)
            wc_sb, bnt_sb = [], []
            for ci, (c0, cp) in enumerate(CCH):
                t = consts.tile([cp, 27], F32, tag=f"wc{ci}", name=f"wc{ci}")
                nc.sync.dma_start(t[:], wconv_d[c0:c0 + cp, :])
                wc_sb.append(t)
                t2 = consts.tile([cp, 3], F32, tag=f"bnt{ci}", name=f"bnt{ci}")
                nc.sync.dma_start(t2[:], bnt_d[c0:c0 + cp, :])
                bnt_sb.append(t2)
            wpa0 = consts.tile([128, CO], BF16, tag="wpa0", name="wpa0")
            nc.sync.dma_start(wpa0[:], wpa_d[0:128, :])
            wpa1 = consts.tile([65, CO], BF16, tag="wpa1", name="wpa1")
            nc.sync.dma_start(wpa1[:], wpa_d[128:193, :])

            # persistent per-tblk vaug tiles [128, 3*65]; ones cols preset
            vaug = [vap.tile([128, 3 * 65], BF16, tag=f"va{tb}", name=f"va{tb}")
                    for tb in range(len(TBLK))]
            for tb, (t0, tn) in enumerate(TBLK):
                ones3 = vaug[tb][:].rearrange("p (h d) -> p h d", h=3)[
                    0:tn, :, 64:65]
                nc.vector.memset(ones3, 1.0)
            aT0 = atp.tile([128, T], BF16, tag="aT0", name="aT0")
            aT1 = atp.tile([65, T], BF16, tag="aT1", name="aT1")
            nc.vector.memset(aT1[64:65, :], 1.0)

            for b in range(BPC):
                # ---- DMA-transposed load: xT bf16 [192, 800] in 2 tiles ----
                # xlo rows = c 0:128; xhi rows 64:128 = c 128:192
                xlo = xTp.tile([128, TP], BF16, tag="xlo", name="xlo")
                xhi = xTp.tile([128, TP], BF16, tag="xhi", name="xhi")
                nc.sync.dma_start(xlo[:], x_d[b, :, 0:128], transpose=True)
                nc.sync.dma_start(xhi[:], x_d[b, :, 64:192], transpose=True)
                xT = [xlo, xhi]  # chunk ci: xT[ci][coff(ci):coff(ci)+cp]

                def cview(ci, lo, hi):
                    """xT rows for channel-chunk ci, sliced [lo:hi] of cp."""
                    base = 0 if ci == 0 else 64
                    return xT[ci][base + lo:base + hi]

                # ---- padded image (shared by q/k/v convs), bf16 ----
                pads = []
                for ci, (c0, cp) in enumerate(CCH):
                    pad = padp.tile([cp, 900], BF16, tag=f"pad{ci}", name=f"pad{ci}")
                    nc.vector.memset(pad[:], 0.0)
                    nc.vector.tensor_copy(
                        _conv_shift_ap(pad[:], 1, 1),
                        _img3(cview(ci, 0, cp)[:, 1:T]))
                    pads.append(pad)

                # ---- depthwise conv + BN -> y (bf16), cls col prepended ----
                ys = []  # [i][chunk]
                for i in range(3):
                    row = []
                    for ci, (c0, cp) in enumerate(CCH):
                        y = yp.tile([cp, T], BF16, tag=f"y{i}{ci}", name=f"y{i}{ci}")
                        acc = accp.tile([cp, 784], BF16, tag=f"acc{ci}", name=f"acc{ci}")
                        acc3 = _img3(acc[:])
                        y3 = _img3(y[:, 1:T])
                        for tap in range(9):
                            dy, dx = tap // 3, tap % 3
                            sh = _conv_shift_ap(pads[ci][:], dy, dx)
                            wcol = wc_sb[ci][:, i * 9 + tap:i * 9 + tap + 1]
                            if tap == 0:
                                nc.vector.tensor_scalar(
                                    acc3, sh, wcol, bnt_sb[ci][:, i:i + 1],
                                    OP.mult, OP.add)
                            elif tap < 8:
                                nc.vector.scalar_tensor_tensor(
                                    acc3, sh, wcol, acc3, OP.mult, OP.add)
                            else:
                                nc.vector.scalar_tensor_tensor(
                                    y3, sh, wcol, acc3, OP.mult, OP.add)
                        nc.vector.tensor_copy(y[:, 0:1], cview(ci, 0, cp)[:, 0:1])
                        row.append(y)
                    ys.append(row)

                # ---- q,k feature-major projections -> qT,kT bf16 ----
                qkT = []  # [i][chunk]
                for i in range(2):
                    row = []
                    for ob, (o0, osz) in enumerate(CCH):
                        ps = psA.tile([128, T], F32, tag="mm", name="mm")
                        for (n0, nn) in NSEG:
                            for ci in range(2):
                                nc.tensor.matmul(
                                    ps[0:osz, n0:n0 + nn],
                                    wq_sb[i][ci][:, o0:o0 + osz],
                                    ys[i][ci][:, n0:n0 + nn],
                                    start=(ci == 0), stop=(ci == 1))
                        dst = qkp.tile([osz, T], BF16, tag=f"qk{i}{ob}", name=f"qk{i}{ob}")
                        nc.vector.tensor_copy(dst[:], ps[0:osz, 0:T])
                        row.append(dst)
                    qkT.append(row)

                def head_rows(qk, h):
                    """[64, T] slice of qT/kT chunks for head h."""
                    if h < 2:
                        return qk[0][h * 64:(h + 1) * 64, :]
                    return qk[1][0:64, :]

                # ---- v token-major -> per-tblk vaug (3 heads fused) ----
                for tb, (t0, tn) in enumerate(TBLK):
                    ps = psA.tile([128, T], F32, tag="mm", name="mm")
                    for ci in range(2):
                        nc.tensor.matmul(
                            ps[0:tn, 0:CO],
                            ys[2][ci][:, t0:t0 + tn],
                            wq_sb[2][ci][:],
                            start=(ci == 0), stop=(ci == 1))
                    dst3 = vaug[tb][:].rearrange("p (h d) -> p h d", h=3)[
                        0:tn, :, 0:64]
                    nc.scalar.copy(
                        dst3, ps[0:tn, 0:CO].rearrange("p (h d) -> p h d", h=3))

                # ---- attention per head; scores(t+1) emitted before PV(t)
                # to keep PE busy while exp(t) runs on ACT ----
                for h in range(NH):
                    kh = head_rows(qkT[1], h)
                    qh = head_rows(qkT[0], h)
                    vh = [vaug[tb][:, h * 65:(h + 1) * 65] for tb in range(len(TBLK))]
                    pv = psA.tile([128, T], F32, tag="mm", name="mm")
                    es_ = [None] * len(TBLK)

                    def emit_scores(tb):
                        t0, tn = TBLK[tb]
                        ss = psA.tile([128, T], F32, tag="mm", name="mm")
                        for (n0, nn) in NSEG:
                            nc.tensor.matmul(
                                ss[0:tn, n0:n0 + nn],
                                kh[:, t0:t0 + tn], qh[:, n0:n0 + nn],
                                start=True, stop=True)
                        e = ep.tile([128, T], BF16, tag="E", name="E")
                        nc.scalar.activation(e[0:tn, 0:T], ss[0:tn, 0:T],
                                             AF.Exp, scale=SCALE)
                        es_[tb] = e

                    def emit_pv(tb):
                        t0, tn = TBLK[tb]
                        for (n0, nn) in NSEG:
                            nc.tensor.matmul(
                                pv[0:65, n0:n0 + nn],
                                vh[tb][0:tn, :],
                                es_[tb][0:tn, n0:n0 + nn],
                                start=(tb == 0), stop=(tb == len(TBLK) - 1))

                    emit_scores(0)
                    for tb in range(len(TBLK)):
                        if tb + 1 < len(TBLK):
                            emit_scores(tb + 1)
                        emit_pv(tb)

                    r = smallp.tile([1, T], F32, tag="r", name="r")
                    nc.vector.reciprocal(r[0:1, :], pv[64:65, 0:T])
                    rb = smallp.tile([64, T], F32, tag="rb", name="rb")
                    nc.gpsimd.partition_broadcast(rb[:], r[0:1, :])
                    dst = aT0[h * 64:(h + 1) * 64, :] if h < 2 else aT1[0:64, :]
                    nc.vector.tensor_tensor(
                        dst, pv[0:64, 0:T], rb[:], OP.mult)

                # ---- final projection (bias via ones row) + store ----
                obuf = op_.tile([128, 6 * CO], BF16, tag="obuf", name="obuf")
                otl = op_.tile([17, CO], BF16, tag="otl", name="otl")
                for tb, (t0, tn) in enumerate(TBLK):
                    fp = psA.tile([128, T], F32, tag="mm", name="mm")
                    nc.tensor.matmul(fp[0:tn, 0:CO], aT0[:, t0:t0 + tn],
                                     wpa0[:], start=True, stop=False)
                    nc.tensor.matmul(fp[0:tn, 0:CO], aT1[:, t0:t0 + tn],
                                     wpa1[:], start=False, stop=True)
                    dst = obuf[:, tb * CO:tb * CO + CO] if tb < 6 else otl[:]
                    nc.scalar.copy(dst[0:tn, :], fp[0:tn, 0:CO])
                nc.sync.dma_start(
                    out_d[b, 0:768, :].rearrange("(n p) c -> p n c", p=128),
                    obuf[:].rearrange("p (n c) -> p n c", n=6, c=CO))
                nc.sync.dma_start(out_d[b, 768:785, :], otl[:])
    if not nc.is_finalized():
        nc.finalize()
    return nc


_NC_CACHE = None


def kernel(**inputs):
    global _NC_CACHE
    x = np.asarray(inputs["x"], dtype=np.float32)
    conv_w = np.asarray(inputs["conv_w"], dtype=np.float32)  # [3,C,1,3,3]
    bn_scale = np.asarray(inputs["bn_scale"], dtype=np.float32)
    bn_bias = np.asarray(inputs["bn_bias"], dtype=np.float32)
    bn_mean = np.asarray(inputs["bn_mean"], dtype=np.float32)
    bn_var = np.asarray(inputs["bn_var"], dtype=np.float32)
    w_qkv = np.asarray(inputs["w_qkv"], dtype=np.float32)  # [3,CO,C]
    w_proj = np.asarray(inputs["w_proj"], dtype=np.float32)  # [CO,CO]
    b_proj = np.asarray(inputs["b_proj"], dtype=np.float32)  # [CO]

    # fold BN into conv taps: y = conv(x, w)*s + (b - mu*s)
    s = bn_scale / np.sqrt(bn_var + BN_EPS)  # [3,C]
    wtap = (conv_w[:, :, 0, :, :].reshape(3, C, 9)
            * s[:, :, None]).astype(np.float32)  # [3,C,9]
    # [C, 27] with column i*9+tap
    wconv_h = np.ascontiguousarray(
        wtap.transpose(1, 0, 2).reshape(C, 27))
    bnt_h = np.ascontiguousarray(
        (bn_bias - bn_mean * s).T).astype(np.float32)  # [C,3]
    wqkvT_h = np.ascontiguousarray(
        w_qkv.transpose(0, 2, 1)).astype(ml_dtypes.bfloat16)  # [3,C,CO]
    wpa_h = np.concatenate(
        [w_proj.T, b_proj[None, :]], axis=0).astype(ml_dtypes.bfloat16)

    if _NC_CACHE is None:
        _NC_CACHE = build_bass()
    nc = _NC_CACHE

    # bf16 x, token dim padded to TP=800 for the XBAR DMA transpose
    xpad = np.zeros((NCORES, BPC, TP, C), dtype=ml_dtypes.bfloat16)
    xpad[:, :, 0:T, :] = x.reshape(NCORES, BPC, T, C).astype(ml_dtypes.bfloat16)
    in_maps = [
        {"x": np.ascontiguousarray(xpad[c]), "wqkvT": wqkvT_h,
         "wconv": wconv_h, "bnt": bnt_h, "wpa": wpa_h}
        for c in range(NCORES)
    ]
    res = run_bass_kernel_spmd(nc, in_maps, list(range(NCORES)), **RUN_KWARGS)
    global LAST_RESULTS
    LAST_RESULTS = res
    out = np.concatenate([np.asarray(r["out"]) for r in res.results], axis=0)
    return out.reshape(B, T, CO).astype(np.float32)


RUN_KWARGS = {}
LAST_RESULTS = None
